# revision 13
# baseline (speedup 1.0000x reference)
"""DKVMN (DeepIRT) forward pass on 8 Trainium2 NeuronCores.

Strategy (v6)
-------------
Pure data parallel over the batch (2048 -> 256 per core, 2 partition-tiles
of 128, kept as one merged [P, 2, M, V] state). Host precomputes gather
tables (weight-only preprocessing):

  wh[q]  = (wdup | hq | w): softmax attention w (pair-duplicated + plain)
           and the query half of the MLP hidden layer
  ea[qa] = (-e | a | -1/e): erase gate (negated), add vector, neg-recip

Per step t (per core):
  NWEA   = w (x) (-e | a)     tile0 built on DVE (pair-broadcast TT),
                              tile1 built on ACT (50 scaled copies,
                              double-buffered, issued one step ahead)
  X2n    = Mv * NWEA.e        in-place over the -w*e field (= -Mv*w*e)
  Mv    += X2n                erase applied
  read   = (-1/e) * sum_m X2n column-sums on the idle TensorE: chained
                              identity-weight matmuls accumulate into
                              PSUM fp32; DVE does a tiny reduce + scale
  Mv    += NWEA.a             add vector applied

The prediction MLP is batched per 4-step chunk on PE/ACT (transposes +
matmuls + tanh/sigmoid). GPSIMD does nothing but the gather DMAs: its
tensor ops contend with the DVE SBUF port and were the v2 bottleneck.
"""

import os
import sys

for _p in ("/root/.axon_site/_ro/trn_rl_repo", "/opt/trn_rl_repo"):
    if os.path.isdir(_p) and _p not in sys.path:
        sys.path.append(_p)

import numpy as np

import concourse.bacc as bacc
import concourse.tile as tile
from concourse import mybir
from concourse.bass_utils import run_bass_kernel_spmd
from concourse.masks import make_identity

# Problem shapes (hardcoded per harness contract)
B, S, M, V, KD, FC = 2048, 200, 50, 200, 50, 50
NQ, NQA = 5001, 10001
NCORES = 8
BL = B // NCORES      # 256 batch rows per core
P = 128               # SBUF partitions
NT = BL // P          # 2 batch tiles per core (merged in one state tensor)
KSTEPS = 2            # time steps per gather block
EAW = 640             # ea-table row: (-e)[0:200] | a[200:400] | (-1/e)[400:600]
WHW = 256             # wh-table row: wdup[0:100] | hq[128:178] | w[192:242]
IDX_PER_BLK = BL * KSTEPS
IDXCOLS = BL * S // 16
KC = 4                # MLP chunk length (steps); S % KC == 0
MG = 2                # m-slots per PE accumulation chain group
JA = 8                # tile0 build slots (m >= M-JA) built by ACT, not DVE

_prog_cache = {}


def _build_program(steps=S):
    dt = mybir.dt
    nc = bacc.Bacc("TRN2", debug=False)

    ea_t = nc.dram_tensor("ea_table", [NQA, EAW], dt.float16, kind="ExternalInput")
    wh_t = nc.dram_tensor("wh_table", [NQ, WHW], dt.float16, kind="ExternalInput")
    w1r_d = nc.dram_tensor("w1r", [2, 100, FC], dt.float16, kind="ExternalInput")
    w2_d = nc.dram_tensor("w2rep", [P, FC], dt.float16, kind="ExternalInput")
    b2_d = nc.dram_tensor("b2rep", [P, 1], dt.float32, kind="ExternalInput")
    mv_d = nc.dram_tensor("mv_init", [1, M * V], dt.float16, kind="ExternalInput")
    qi_d = nc.dram_tensor("qidx", [P, IDXCOLS], dt.int16, kind="ExternalInput")
    qa_d = nc.dram_tensor("qaidx", [P, IDXCOLS], dt.int16, kind="ExternalInput")
    preds_d = nc.dram_tensor("preds_out", [BL, S], dt.float32, kind="ExternalOutput")

    nblk = steps // KSTEPS

    from contextlib import ExitStack

    with tile.TileContext(nc) as tc, ExitStack() as ctx:
        consts = ctx.enter_context(tc.tile_pool(name="consts", bufs=1))
        state = ctx.enter_context(tc.tile_pool(name="state", bufs=1))
        gath = ctx.enter_context(tc.tile_pool(name="gath", bufs=2))
        small = ctx.enter_context(tc.tile_pool(name="small", bufs=2))
        psum = ctx.enter_context(tc.tile_pool(name="psum", bufs=2, space="PSUM"))
        psmm = ctx.enter_context(tc.tile_pool(name="psmm", bufs=1, space="PSUM"))
        psrd = ctx.enter_context(tc.tile_pool(name="psrd", bufs=2, space="PSUM"))

        mult = mybir.AluOpType.mult
        addop = mybir.AluOpType.add

        # ---- constants ----
        w1r_sb = consts.tile([100, 2, FC], dt.float16)
        for c in range(2):
            nc.sync.dma_start(out=w1r_sb[:, c, :], in_=w1r_d[c])
        w2_sb = consts.tile([P, FC], dt.float16)
        nc.sync.dma_start(out=w2_sb[:], in_=w2_d[:])
        b2_sb = consts.tile([P, 1], dt.float32)
        nc.sync.dma_start(out=b2_sb[:], in_=b2_d[:])
        ident = consts.tile([P, P], dt.float16)
        make_identity(nc, ident)

        # ---- persistent state ----
        Mv = state.tile([P, NT, M, V], dt.float16, tag="mv", name="Mv")
        for tl in range(NT):
            nc.sync.dma_start(
                out=Mv[:, tl].rearrange("p m v -> p (m v)"),
                in_=mv_d[:].to_broadcast((P, M * V)),
            )
        NW0 = state.tile([P, M, 2 * V], dt.float16, tag="nw0", name="NW0")
        NW1 = state.tile([P, 2, M, 2 * V], dt.float16, tag="nw1", name="NW1")
        RE = state.tile([P, NT, V], dt.float32, tag="re", name="RE")
        RB = state.tile([P, NT, 2, KC, V], dt.float16, tag="rb", name="RB")
        HQ = state.tile([P, NT, 2, KC, FC], dt.float16, tag="hq", name="HQ")
        RT = state.tile([100, KC, 2, P], dt.float16, tag="rt", name="RT")
        WF = state.tile([P, 2, NT * KSTEPS, M], dt.float32, tag="wf", name="WF")
        preds_buf = state.tile([P, NT, S], dt.float32, tag="preds")

        def flush_pe(c0, klen, cpar):
            """PE part of the batched MLP: transposes + matmuls into PSUM."""
            hps = []
            for tl in range(NT):
                h_ps = psmm.tile([P, KC, FC], dt.float32, tag=f"hps{tl}",
                                 name=f"hps{tl}")
                for kk in range(klen):
                    for h in range(2):
                        pT = psum.tile([100, P], dt.float16, tag="pT", name="pT")
                        nc.tensor.transpose(
                            pT[:], RB[:, tl, cpar, kk, h * 100:(h + 1) * 100],
                            ident[:])
                        nc.scalar.copy(RT[:, kk, h, :], pT[:])
                    nc.tensor.matmul(h_ps[:, kk, :], lhsT=RT[:, kk, 0, :],
                                     rhs=w1r_sb[:, 0, :], start=True, stop=False)
                    nc.tensor.matmul(h_ps[:, kk, :], lhsT=RT[:, kk, 1, :],
                                     rhs=w1r_sb[:, 1, :], start=False, stop=True)
                hps.append(h_ps)
            return hps

        def flush_dve(c0, klen, cpar, hps):
            """DVE/ACT tail of the batched MLP (issued a step later)."""
            for tl in range(NT):
                h_ps = hps[tl]
                hh = small.tile([P, KC, FC], dt.float16, tag="hh", name="hh")
                nc.vector.tensor_add(hh[:, 0:klen, :], h_ps[:, 0:klen, :],
                                     HQ[:, tl, cpar, 0:klen, :])
                nc.scalar.activation(hh[:, 0:klen, :], hh[:, 0:klen, :],
                                     mybir.ActivationFunctionType.Tanh)
                hw2 = small.tile([P, KC, FC], dt.float16, tag="hw2", name="hw2")
                nc.vector.tensor_mul(
                    hw2[:, 0:klen, :], hh[:, 0:klen, :],
                    w2_sb[:, None, :].to_broadcast((P, klen, FC)))
                pacc = small.tile([P, KC], dt.float32, tag="pacc", name="pacc")
                nc.vector.tensor_reduce(pacc[:, 0:klen], hw2[:, 0:klen, :],
                                        mybir.AxisListType.X, addop)
                nc.scalar.activation(
                    preds_buf[:, tl, c0:c0 + klen], pacc[:, 0:klen],
                    mybir.ActivationFunctionType.Sigmoid, bias=b2_sb[:],
                )

        def issue_gather(g):
            qi = gath.tile([P, IDX_PER_BLK // 16], dt.int16, tag="qi", name="qi")
            qa = gath.tile([P, IDX_PER_BLK // 16], dt.int16, tag="qa", name="qa")
            c0 = g * (IDX_PER_BLK // 16)
            nc.sync.dma_start(out=qi[:], in_=qi_d[:, c0:c0 + IDX_PER_BLK // 16])
            nc.sync.dma_start(out=qa[:], in_=qa_d[:, c0:c0 + IDX_PER_BLK // 16])
            ea_blk = gath.tile([P, NT * KSTEPS, EAW], dt.float16, tag="ea", name="ea")
            wh_blk = gath.tile([P, NT * KSTEPS, WHW], dt.float16, tag="wh", name="wh")
            nc.gpsimd.dma_gather(ea_blk[:], ea_t[:], qa[:], IDX_PER_BLK, IDX_PER_BLK, EAW)
            nc.gpsimd.dma_gather(wh_blk[:], wh_t[:], qi[:], IDX_PER_BLK, IDX_PER_BLK, WHW)
            return ea_blk, wh_blk

        def block_prep(g, gpar, ea_blk, wh_blk):
            # fp32 w copies for ACT build scales (all 4 block rows)
            nc.scalar.copy(WF[:, gpar, :, :], wh_blk[:, :, 192:192 + M])

        def act_build_nw1(spar, ea_row, wf_scale):
            """ACT: NW1[spar][m, 0:400] = w[m] * (-e | a) for tile1."""
            for m in range(M):
                nc.scalar.activation(
                    NW1[:, spar, m, 0:2 * V], ea_row,
                    mybir.ActivationFunctionType.Copy,
                    scale=wf_scale[:, m:m + 1])

        def act_build_nw0j(ea_row, wf_scale):
            """ACT: tile0's last JA field rows (DVE builds the rest)."""
            for m in range(M - JA, M):
                nc.scalar.activation(
                    NW0[:, m, 0:2 * V], ea_row,
                    mybir.ActivationFunctionType.Copy,
                    scale=wf_scale[:, m:m + 1])

        # ---- prologue: first gather + first ACT builds (step 0) ----
        cur = issue_gather(0)
        block_prep(0, 0, *cur)
        act_build_nw1(0, cur[0][:, 1, 0:2 * V], WF[:, 0, 1, :])
        act_build_nw0j(cur[0][:, 0, 0:2 * V], WF[:, 0, 0, :])

        pending = None
        nxt = cur
        for g in range(nblk):
            ea_blk, wh_blk = nxt
            gpar = g % 2
            if g + 1 < nblk:
                nxt = issue_gather(g + 1)

            for k in range(KSTEPS):
                t = g * KSTEPS + k
                kk, cpar, spar = t % KC, (t // KC) % 2, t % 2
                c0t, c1t = 2 * k, 2 * k + 1  # block row ids for tile0/tile1

                # HQ rows for the chunked MLP (both tiles, via idle DMA engine)
                wrows = wh_blk[:].rearrange("p (kx t) w -> p kx t w", t=2)
                nc.sync.dma_start(out=HQ[:, :, cpar, kk, :],
                                  in_=wrows[:, k, :, 128:128 + FC])

                # --- DVE: build tile0's NWEA = w (x) (-e | a), m < M-JA ---
                MD = M - JA
                wp = wh_blk[:, c0t, 0:2 * M].rearrange("p (m two) -> p m two", two=2)
                wp_bc = wp[:, 0:MD, None, :].to_broadcast((P, MD, V, 2))
                nea = ea_blk[:, c0t, 0:2 * V].rearrange(
                    "p (vh two) -> p vh two", two=2)
                nea_bc = nea[:, None, :, :].to_broadcast((P, MD, V, 2))
                nw0v = NW0[:, 0:MD, :].rearrange(
                    "p m (vh two) -> p m vh two", two=2)
                nc.vector.tensor_mul(nw0v, wp_bc, nea_bc)

                # --- X2n (in-place over the -w*e fields) ---
                nc.vector.tensor_mul(NW0[:, :, 0:V], NW0[:, :, 0:V], Mv[:, 0])
                nc.vector.tensor_mul(NW1[:, spar, :, 0:V], NW1[:, spar, :, 0:V],
                                     Mv[:, 1])

                # --- PE: column-sums of X2n into PSUM (read numerator) ---
                # [P, NT, 512]: each tile's slab fills one PSUM bank exactly;
                # a matmul accumulation target must not straddle banks.
                readp = psrd.tile([P, NT, 512], dt.float32, tag="readp",
                                  name="readp")
                ngrp = M // MG
                for tl in range(NT):
                    src = NW0 if tl == 0 else NW1[:, spar]
                    out_v = readp[:, tl, 0:MG * V].rearrange(
                        "p (g v) -> p g v", g=MG)
                    for gi in range(ngrp):
                        nc.tensor.matmul(
                            out_v, lhsT=ident[:],
                            rhs=src[:, MG * gi:MG * (gi + 1), 0:V],
                            start=(gi == 0), stop=(gi == ngrp - 1))

                # --- erase + add (plain flat adds) ---
                nc.vector.tensor_add(Mv[:, 0], Mv[:, 0], NW0[:, :, 0:V])
                nc.vector.tensor_add(Mv[:, 1], Mv[:, 1], NW1[:, spar, :, 0:V])
                nc.vector.tensor_add(Mv[:, 0], Mv[:, 0], NW0[:, :, V:2 * V])
                nc.vector.tensor_add(Mv[:, 1], Mv[:, 1], NW1[:, spar, :, V:2 * V])

                # --- ACT: build tile1's NWEA for step t+1 (pingpong) ---
                if t + 1 < steps:
                    kn = (t + 1) % KSTEPS
                    gn = (t + 1) // KSTEPS
                    eab = ea_blk if gn == g else nxt[0]
                    if gn != g:
                        block_prep(gn, gn % 2, *nxt)
                    act_build_nw1(1 - spar, eab[:, 2 * kn + 1, 0:2 * V],
                                  WF[:, gn % 2, 2 * kn + 1, :])
                    act_build_nw0j(eab[:, 2 * kn, 0:2 * V],
                                   WF[:, gn % 2, 2 * kn, :])

                # --- read: reduce PSUM groups + scale by -1/e ---
                rpv = readp[:, :, 0:MG * V].rearrange(
                    "p t (g v) -> p t v g", g=MG)
                nc.vector.tensor_reduce(RE[:], rpv, mybir.AxisListType.X, addop)
                nrec = ea_blk[:].rearrange("p (kx t) w -> p kx t w", t=2)
                nc.vector.tensor_mul(RB[:, :, cpar, kk, :], RE[:],
                                     nrec[:, k, :, 400:400 + V])

                # --- chunked MLP flush ---
                if pending is not None and kk == 1:
                    flush_dve(*pending)
                    pending = None
                if kk == KC - 1:
                    hps = flush_pe(t - KC + 1, KC, cpar)
                    pending = (t - KC + 1, KC, cpar, hps)
        if pending is not None:
            flush_dve(*pending)
            pending = None
        # ---- write out ----
        pv = preds_d[:].rearrange("(n p) s -> n p s", p=P)
        for tl in range(NT):
            nc.sync.dma_start(out=pv[tl][:, 0:steps], in_=preds_buf[:, tl, 0:steps])

    nc.finalize()
    return nc


def _wrap_idx(seq):
    """seq [N] -> [128, N//16] int16 wrapped (idx i at [i%16, i//16], 8x replicated)."""
    n = seq.shape[0]
    arr16 = seq.reshape(n // 16, 16).T.astype(np.int16)
    return np.tile(arr16, (8, 1))


def _host_tables(inputs):
    f32 = np.float32
    qe = inputs["q_embed_w"].astype(f32)
    qae = inputs["qa_embed_w"].astype(f32)
    km = inputs["key_memory"].astype(f32)

    logits = qe @ km.T
    ex = np.exp(logits - logits.max(-1, keepdims=True))
    wsoft = ex / ex.sum(-1, keepdims=True)
    hq = qe @ inputs["pred_w1"][V:, :].astype(f32) + inputs["pred_b1"].astype(f32)
    esig = 1.0 / (1.0 + np.exp(-(qae @ inputs["erase_w"].astype(f32)
                                 + inputs["erase_b"].astype(f32))))
    atanh = np.tanh(qae @ inputs["add_w"].astype(f32) + inputs["add_b"].astype(f32))

    ea = np.zeros((NQA, EAW), np.float16)
    ea[:, 0:V] = (-esig).astype(np.float16)
    ea[:, V:2 * V] = atanh.astype(np.float16)
    ea[:, 400:400 + V] = (-1.0 / esig).astype(np.float16)
    wh = np.zeros((NQ, WHW), np.float16)
    wh[:, 0:2 * M] = np.repeat(wsoft.astype(np.float16), 2, axis=1)
    wh[:, 128:128 + FC] = hq.astype(np.float16)
    wh[:, 192:192 + M] = wsoft.astype(np.float16)

    w1r = inputs["pred_w1"][:V, :].astype(np.float16).reshape(2, 100, FC)
    w2rep = np.tile(inputs["pred_w2"][:, 0].astype(np.float16)[None, :], (P, 1))
    b2rep = np.full((P, 1), inputs["pred_b2"][0], np.float32)
    mv_init = inputs["init_value_memory"].astype(np.float16).reshape(1, -1)
    return dict(ea_table=ea, wh_table=wh, w1r=w1r, w2rep=w2rep, b2rep=b2rep,
                mv_init=mv_init)


def kernel(**inputs):
    inputs = {k: np.asarray(v) for k, v in inputs.items()}
    steps = int(os.environ.get("KERNEL_STEPS", S))

    if steps not in _prog_cache:
        _prog_cache[steps] = _build_program(steps)
    nc = _prog_cache[steps]

    shared = _host_tables(inputs)
    q = inputs["q_data"].astype(np.int64)
    qa = inputs["qa_data"].astype(np.int64)

    in_maps = []
    for core in range(NCORES):
        qs = q[core * BL:(core + 1) * BL]       # [256, S]
        qas = qa[core * BL:(core + 1) * BL]
        # gather order: block g, step k, tile tl, partition p
        def order(x):
            xt = x.T.reshape(S, NT, P)
            return xt.reshape(S // KSTEPS, KSTEPS, NT, P).reshape(-1)
        m = dict(shared)
        m["qidx"] = _wrap_idx(order(qs))
        m["qaidx"] = _wrap_idx(order(qas))
        in_maps.append(m)

    trace = bool(int(os.environ.get("KERNEL_TRACE", "0")))
    res = run_bass_kernel_spmd(nc, in_maps, core_ids=list(range(NCORES)), trace=trace)
    global LAST_RESULTS
    LAST_RESULTS = res
    preds = np.concatenate(
        [res.results[i]["preds_out"] for i in range(NCORES)], axis=0
    ).astype(np.float32)
    z = np.zeros_like(preds)
    return (preds, z, z, z)


# revision 14
# speedup vs baseline: 1.0044x; 1.0044x over previous
"""DKVMN (DeepIRT) forward pass on 8 Trainium2 NeuronCores.

Strategy (v6)
-------------
Pure data parallel over the batch (2048 -> 256 per core, 2 partition-tiles
of 128, kept as one merged [P, 2, M, V] state). Host precomputes gather
tables (weight-only preprocessing):

  wh[q]  = (wdup | hq | w): softmax attention w (pair-duplicated + plain)
           and the query half of the MLP hidden layer
  ea[qa] = (-e | a | -1/e): erase gate (negated), add vector, neg-recip

Per step t (per core):
  NWEA   = w (x) (-e | a)     tile0 built on DVE (pair-broadcast TT),
                              tile1 built on ACT (50 scaled copies,
                              double-buffered, issued one step ahead)
  X2n    = Mv * NWEA.e        in-place over the -w*e field (= -Mv*w*e)
  Mv    += X2n                erase applied
  read   = (-1/e) * sum_m X2n column-sums on the idle TensorE: chained
                              identity-weight matmuls accumulate into
                              PSUM fp32; DVE does a tiny reduce + scale
  Mv    += NWEA.a             add vector applied

The prediction MLP is batched per 4-step chunk on PE/ACT (transposes +
matmuls + tanh/sigmoid). GPSIMD does nothing but the gather DMAs: its
tensor ops contend with the DVE SBUF port and were the v2 bottleneck.
"""

import os
import sys

for _p in ("/root/.axon_site/_ro/trn_rl_repo", "/opt/trn_rl_repo"):
    if os.path.isdir(_p) and _p not in sys.path:
        sys.path.append(_p)

import numpy as np

import concourse.bacc as bacc
import concourse.tile as tile
from concourse import mybir
from concourse.bass_utils import run_bass_kernel_spmd
from concourse.masks import make_identity

# Problem shapes (hardcoded per harness contract)
B, S, M, V, KD, FC = 2048, 200, 50, 200, 50, 50
NQ, NQA = 5001, 10001
NCORES = 8
BL = B // NCORES      # 256 batch rows per core
P = 128               # SBUF partitions
NT = BL // P          # 2 batch tiles per core (merged in one state tensor)
KSTEPS = 2            # time steps per gather block
EAW = 640             # ea-table row: (-e)[0:200] | a[200:400] | (-1/e)[400:600]
WHW = 256             # wh-table row: wdup[0:100] | hq[128:178] | w[192:242]
IDX_PER_BLK = BL * KSTEPS
IDXCOLS = BL * S // 16
KC = 4                # MLP chunk length (steps); S % KC == 0
MG = 2                # m-slots per PE accumulation chain group
JA = 6                # tile0 build slots (m >= M-JA) built by ACT, not DVE

_prog_cache = {}


def _build_program(steps=S):
    dt = mybir.dt
    nc = bacc.Bacc("TRN2", debug=False)

    ea_t = nc.dram_tensor("ea_table", [NQA, EAW], dt.float16, kind="ExternalInput")
    wh_t = nc.dram_tensor("wh_table", [NQ, WHW], dt.float16, kind="ExternalInput")
    w1r_d = nc.dram_tensor("w1r", [2, 100, FC], dt.float16, kind="ExternalInput")
    w2_d = nc.dram_tensor("w2rep", [P, FC], dt.float16, kind="ExternalInput")
    b2_d = nc.dram_tensor("b2rep", [P, 1], dt.float32, kind="ExternalInput")
    mv_d = nc.dram_tensor("mv_init", [1, M * V], dt.float16, kind="ExternalInput")
    qi_d = nc.dram_tensor("qidx", [P, IDXCOLS], dt.int16, kind="ExternalInput")
    qa_d = nc.dram_tensor("qaidx", [P, IDXCOLS], dt.int16, kind="ExternalInput")
    preds_d = nc.dram_tensor("preds_out", [BL, S], dt.float32, kind="ExternalOutput")

    nblk = steps // KSTEPS

    from contextlib import ExitStack

    with tile.TileContext(nc) as tc, ExitStack() as ctx:
        consts = ctx.enter_context(tc.tile_pool(name="consts", bufs=1))
        state = ctx.enter_context(tc.tile_pool(name="state", bufs=1))
        gath = ctx.enter_context(tc.tile_pool(name="gath", bufs=2))
        small = ctx.enter_context(tc.tile_pool(name="small", bufs=2))
        psum = ctx.enter_context(tc.tile_pool(name="psum", bufs=2, space="PSUM"))
        psmm = ctx.enter_context(tc.tile_pool(name="psmm", bufs=1, space="PSUM"))
        psrd = ctx.enter_context(tc.tile_pool(name="psrd", bufs=2, space="PSUM"))

        mult = mybir.AluOpType.mult
        addop = mybir.AluOpType.add

        # ---- constants ----
        w1r_sb = consts.tile([100, 2, FC], dt.float16)
        for c in range(2):
            nc.sync.dma_start(out=w1r_sb[:, c, :], in_=w1r_d[c])
        w2_sb = consts.tile([P, FC], dt.float16)
        nc.sync.dma_start(out=w2_sb[:], in_=w2_d[:])
        b2_sb = consts.tile([P, 1], dt.float32)
        nc.sync.dma_start(out=b2_sb[:], in_=b2_d[:])
        ident = consts.tile([P, P], dt.float16)
        make_identity(nc, ident)

        # ---- persistent state ----
        Mv = state.tile([P, NT, M, V], dt.float16, tag="mv", name="Mv")
        for tl in range(NT):
            nc.sync.dma_start(
                out=Mv[:, tl].rearrange("p m v -> p (m v)"),
                in_=mv_d[:].to_broadcast((P, M * V)),
            )
        NW0 = state.tile([P, M, 2 * V], dt.float16, tag="nw0", name="NW0")
        NW1 = state.tile([P, 2, M, 2 * V], dt.float16, tag="nw1", name="NW1")
        RE = state.tile([P, NT, V], dt.float32, tag="re", name="RE")
        RB = state.tile([P, NT, 2, KC, V], dt.float16, tag="rb", name="RB")
        HQ = state.tile([P, NT, 2, KC, FC], dt.float16, tag="hq", name="HQ")
        RT = state.tile([100, KC, 2, P], dt.float16, tag="rt", name="RT")
        WF = state.tile([P, 2, NT * KSTEPS, M], dt.float32, tag="wf", name="WF")
        preds_buf = state.tile([P, NT, S], dt.float32, tag="preds")

        def flush_pe(c0, klen, cpar):
            """PE part of the batched MLP: transposes + matmuls into PSUM."""
            hps = []
            for tl in range(NT):
                h_ps = psmm.tile([P, KC, FC], dt.float32, tag=f"hps{tl}",
                                 name=f"hps{tl}")
                for kk in range(klen):
                    for h in range(2):
                        pT = psum.tile([100, P], dt.float16, tag="pT", name="pT")
                        nc.tensor.transpose(
                            pT[:], RB[:, tl, cpar, kk, h * 100:(h + 1) * 100],
                            ident[:])
                        nc.scalar.copy(RT[:, kk, h, :], pT[:])
                    nc.tensor.matmul(h_ps[:, kk, :], lhsT=RT[:, kk, 0, :],
                                     rhs=w1r_sb[:, 0, :], start=True, stop=False)
                    nc.tensor.matmul(h_ps[:, kk, :], lhsT=RT[:, kk, 1, :],
                                     rhs=w1r_sb[:, 1, :], start=False, stop=True)
                hps.append(h_ps)
            return hps

        def flush_dve(c0, klen, cpar, hps):
            """DVE/ACT tail of the batched MLP (issued a step later)."""
            for tl in range(NT):
                h_ps = hps[tl]
                hh = small.tile([P, KC, FC], dt.float16, tag="hh", name="hh")
                nc.vector.tensor_add(hh[:, 0:klen, :], h_ps[:, 0:klen, :],
                                     HQ[:, tl, cpar, 0:klen, :])
                nc.scalar.activation(hh[:, 0:klen, :], hh[:, 0:klen, :],
                                     mybir.ActivationFunctionType.Tanh)
                hw2 = small.tile([P, KC, FC], dt.float16, tag="hw2", name="hw2")
                nc.vector.tensor_mul(
                    hw2[:, 0:klen, :], hh[:, 0:klen, :],
                    w2_sb[:, None, :].to_broadcast((P, klen, FC)))
                pacc = small.tile([P, KC], dt.float32, tag="pacc", name="pacc")
                nc.vector.tensor_reduce(pacc[:, 0:klen], hw2[:, 0:klen, :],
                                        mybir.AxisListType.X, addop)
                nc.scalar.activation(
                    preds_buf[:, tl, c0:c0 + klen], pacc[:, 0:klen],
                    mybir.ActivationFunctionType.Sigmoid, bias=b2_sb[:],
                )

        def issue_gather(g):
            qi = gath.tile([P, IDX_PER_BLK // 16], dt.int16, tag="qi", name="qi")
            qa = gath.tile([P, IDX_PER_BLK // 16], dt.int16, tag="qa", name="qa")
            c0 = g * (IDX_PER_BLK // 16)
            nc.sync.dma_start(out=qi[:], in_=qi_d[:, c0:c0 + IDX_PER_BLK // 16])
            nc.sync.dma_start(out=qa[:], in_=qa_d[:, c0:c0 + IDX_PER_BLK // 16])
            ea_blk = gath.tile([P, NT * KSTEPS, EAW], dt.float16, tag="ea", name="ea")
            wh_blk = gath.tile([P, NT * KSTEPS, WHW], dt.float16, tag="wh", name="wh")
            nc.gpsimd.dma_gather(ea_blk[:], ea_t[:], qa[:], IDX_PER_BLK, IDX_PER_BLK, EAW)
            nc.gpsimd.dma_gather(wh_blk[:], wh_t[:], qi[:], IDX_PER_BLK, IDX_PER_BLK, WHW)
            return ea_blk, wh_blk

        def block_prep(g, gpar, ea_blk, wh_blk):
            # fp32 w copies for ACT build scales (all 4 block rows)
            nc.scalar.copy(WF[:, gpar, :, :], wh_blk[:, :, 192:192 + M])

        def act_build_nw1(spar, ea_row, wf_scale):
            """ACT: NW1[spar][m, 0:400] = w[m] * (-e | a) for tile1."""
            for m in range(M):
                nc.scalar.activation(
                    NW1[:, spar, m, 0:2 * V], ea_row,
                    mybir.ActivationFunctionType.Copy,
                    scale=wf_scale[:, m:m + 1])

        def act_build_nw0j(ea_row, wf_scale):
            """ACT: tile0's last JA field rows (DVE builds the rest)."""
            for m in range(M - JA, M):
                nc.scalar.activation(
                    NW0[:, m, 0:2 * V], ea_row,
                    mybir.ActivationFunctionType.Copy,
                    scale=wf_scale[:, m:m + 1])

        # ---- prologue: first gather + first ACT builds (step 0) ----
        cur = issue_gather(0)
        block_prep(0, 0, *cur)
        act_build_nw1(0, cur[0][:, 1, 0:2 * V], WF[:, 0, 1, :])
        act_build_nw0j(cur[0][:, 0, 0:2 * V], WF[:, 0, 0, :])

        pending = None
        nxt = cur
        for g in range(nblk):
            ea_blk, wh_blk = nxt
            gpar = g % 2
            if g + 1 < nblk:
                nxt = issue_gather(g + 1)

            for k in range(KSTEPS):
                t = g * KSTEPS + k
                kk, cpar, spar = t % KC, (t // KC) % 2, t % 2
                c0t, c1t = 2 * k, 2 * k + 1  # block row ids for tile0/tile1

                # HQ rows for the chunked MLP (both tiles, one ACT op)
                wrows = wh_blk[:].rearrange("p (kx t) w -> p kx t w", t=2)
                nc.scalar.copy(HQ[:, :, cpar, kk, :], wrows[:, k, :, 128:128 + FC])

                # --- DVE: build tile0's NWEA = w (x) (-e | a), m < M-JA ---
                MD = M - JA
                wp = wh_blk[:, c0t, 0:2 * M].rearrange("p (m two) -> p m two", two=2)
                wp_bc = wp[:, 0:MD, None, :].to_broadcast((P, MD, V, 2))
                nea = ea_blk[:, c0t, 0:2 * V].rearrange(
                    "p (vh two) -> p vh two", two=2)
                nea_bc = nea[:, None, :, :].to_broadcast((P, MD, V, 2))
                nw0v = NW0[:, 0:MD, :].rearrange(
                    "p m (vh two) -> p m vh two", two=2)
                nc.vector.tensor_mul(nw0v, wp_bc, nea_bc)

                # --- X2n (in-place over the -w*e fields) ---
                nc.vector.tensor_mul(NW0[:, :, 0:V], NW0[:, :, 0:V], Mv[:, 0])
                nc.vector.tensor_mul(NW1[:, spar, :, 0:V], NW1[:, spar, :, 0:V],
                                     Mv[:, 1])

                # --- PE: column-sums of X2n into PSUM (read numerator) ---
                # [P, NT, 512]: each tile's slab fills one PSUM bank exactly;
                # a matmul accumulation target must not straddle banks.
                readp = psrd.tile([P, NT, 512], dt.float32, tag="readp",
                                  name="readp")
                ngrp = M // MG
                for tl in range(NT):
                    src = NW0 if tl == 0 else NW1[:, spar]
                    out_v = readp[:, tl, 0:MG * V].rearrange(
                        "p (g v) -> p g v", g=MG)
                    for gi in range(ngrp):
                        nc.tensor.matmul(
                            out_v, lhsT=ident[:],
                            rhs=src[:, MG * gi:MG * (gi + 1), 0:V],
                            start=(gi == 0), stop=(gi == ngrp - 1))

                # --- erase + add (plain flat adds) ---
                nc.vector.tensor_add(Mv[:, 0], Mv[:, 0], NW0[:, :, 0:V])
                nc.vector.tensor_add(Mv[:, 1], Mv[:, 1], NW1[:, spar, :, 0:V])
                nc.vector.tensor_add(Mv[:, 0], Mv[:, 0], NW0[:, :, V:2 * V])
                nc.vector.tensor_add(Mv[:, 1], Mv[:, 1], NW1[:, spar, :, V:2 * V])

                # --- ACT: build tile1's NWEA for step t+1 (pingpong) ---
                if t + 1 < steps:
                    kn = (t + 1) % KSTEPS
                    gn = (t + 1) // KSTEPS
                    eab = ea_blk if gn == g else nxt[0]
                    if gn != g:
                        block_prep(gn, gn % 2, *nxt)
                    act_build_nw1(1 - spar, eab[:, 2 * kn + 1, 0:2 * V],
                                  WF[:, gn % 2, 2 * kn + 1, :])
                    act_build_nw0j(eab[:, 2 * kn, 0:2 * V],
                                   WF[:, gn % 2, 2 * kn, :])

                # --- read: reduce PSUM groups + scale by -1/e ---
                rpv = readp[:, :, 0:MG * V].rearrange(
                    "p t (g v) -> p t v g", g=MG)
                nc.vector.tensor_reduce(RE[:], rpv, mybir.AxisListType.X, addop)
                nrec = ea_blk[:].rearrange("p (kx t) w -> p kx t w", t=2)
                nc.vector.tensor_mul(RB[:, :, cpar, kk, :], RE[:],
                                     nrec[:, k, :, 400:400 + V])

                # --- chunked MLP flush ---
                if pending is not None and kk == 1:
                    flush_dve(*pending)
                    pending = None
                if kk == KC - 1:
                    hps = flush_pe(t - KC + 1, KC, cpar)
                    pending = (t - KC + 1, KC, cpar, hps)
        if pending is not None:
            flush_dve(*pending)
            pending = None
        # ---- write out ----
        pv = preds_d[:].rearrange("(n p) s -> n p s", p=P)
        for tl in range(NT):
            nc.sync.dma_start(out=pv[tl][:, 0:steps], in_=preds_buf[:, tl, 0:steps])

    nc.finalize()
    return nc


def _wrap_idx(seq):
    """seq [N] -> [128, N//16] int16 wrapped (idx i at [i%16, i//16], 8x replicated)."""
    n = seq.shape[0]
    arr16 = seq.reshape(n // 16, 16).T.astype(np.int16)
    return np.tile(arr16, (8, 1))


def _host_tables(inputs):
    f32 = np.float32
    qe = inputs["q_embed_w"].astype(f32)
    qae = inputs["qa_embed_w"].astype(f32)
    km = inputs["key_memory"].astype(f32)

    logits = qe @ km.T
    ex = np.exp(logits - logits.max(-1, keepdims=True))
    wsoft = ex / ex.sum(-1, keepdims=True)
    hq = qe @ inputs["pred_w1"][V:, :].astype(f32) + inputs["pred_b1"].astype(f32)
    esig = 1.0 / (1.0 + np.exp(-(qae @ inputs["erase_w"].astype(f32)
                                 + inputs["erase_b"].astype(f32))))
    atanh = np.tanh(qae @ inputs["add_w"].astype(f32) + inputs["add_b"].astype(f32))

    ea = np.zeros((NQA, EAW), np.float16)
    ea[:, 0:V] = (-esig).astype(np.float16)
    ea[:, V:2 * V] = atanh.astype(np.float16)
    ea[:, 400:400 + V] = (-1.0 / esig).astype(np.float16)
    wh = np.zeros((NQ, WHW), np.float16)
    wh[:, 0:2 * M] = np.repeat(wsoft.astype(np.float16), 2, axis=1)
    wh[:, 128:128 + FC] = hq.astype(np.float16)
    wh[:, 192:192 + M] = wsoft.astype(np.float16)

    w1r = inputs["pred_w1"][:V, :].astype(np.float16).reshape(2, 100, FC)
    w2rep = np.tile(inputs["pred_w2"][:, 0].astype(np.float16)[None, :], (P, 1))
    b2rep = np.full((P, 1), inputs["pred_b2"][0], np.float32)
    mv_init = inputs["init_value_memory"].astype(np.float16).reshape(1, -1)
    return dict(ea_table=ea, wh_table=wh, w1r=w1r, w2rep=w2rep, b2rep=b2rep,
                mv_init=mv_init)


def kernel(**inputs):
    inputs = {k: np.asarray(v) for k, v in inputs.items()}
    steps = int(os.environ.get("KERNEL_STEPS", S))

    if steps not in _prog_cache:
        _prog_cache[steps] = _build_program(steps)
    nc = _prog_cache[steps]

    shared = _host_tables(inputs)
    q = inputs["q_data"].astype(np.int64)
    qa = inputs["qa_data"].astype(np.int64)

    in_maps = []
    for core in range(NCORES):
        qs = q[core * BL:(core + 1) * BL]       # [256, S]
        qas = qa[core * BL:(core + 1) * BL]
        # gather order: block g, step k, tile tl, partition p
        def order(x):
            xt = x.T.reshape(S, NT, P)
            return xt.reshape(S // KSTEPS, KSTEPS, NT, P).reshape(-1)
        m = dict(shared)
        m["qidx"] = _wrap_idx(order(qs))
        m["qaidx"] = _wrap_idx(order(qas))
        in_maps.append(m)

    trace = bool(int(os.environ.get("KERNEL_TRACE", "0")))
    res = run_bass_kernel_spmd(nc, in_maps, core_ids=list(range(NCORES)), trace=trace)
    global LAST_RESULTS
    LAST_RESULTS = res
    preds = np.concatenate(
        [res.results[i]["preds_out"] for i in range(NCORES)], axis=0
    ).astype(np.float32)
    z = np.zeros_like(preds)
    return (preds, z, z, z)


# revision 16
# speedup vs baseline: 1.0240x; 1.0195x over previous
"""DKVMN (DeepIRT) forward pass on 8 Trainium2 NeuronCores.

Strategy (v6)
-------------
Pure data parallel over the batch (2048 -> 256 per core, 2 partition-tiles
of 128, kept as one merged [P, 2, M, V] state). Host precomputes gather
tables (weight-only preprocessing):

  wh[q]  = (wdup | hq | w): softmax attention w (pair-duplicated + plain)
           and the query half of the MLP hidden layer
  ea[qa] = (-e | a | -1/e): erase gate (negated), add vector, neg-recip

Per step t (per core):
  NWEA   = w (x) (-e | a)     tile0 built on DVE (pair-broadcast TT),
                              tile1 built on ACT (50 scaled copies,
                              double-buffered, issued one step ahead)
  X2n    = Mv * NWEA.e        in-place over the -w*e field (= -Mv*w*e)
  Mv    += X2n                erase applied
  read   = (-1/e) * sum_m X2n column-sums on the idle TensorE: chained
                              identity-weight matmuls accumulate into
                              PSUM fp32; DVE does a tiny reduce + scale
  Mv    += NWEA.a             add vector applied

The prediction MLP is batched per 4-step chunk on PE/ACT (transposes +
matmuls + tanh/sigmoid). GPSIMD does nothing but the gather DMAs: its
tensor ops contend with the DVE SBUF port and were the v2 bottleneck.
"""

import os
import sys

for _p in ("/root/.axon_site/_ro/trn_rl_repo", "/opt/trn_rl_repo"):
    if os.path.isdir(_p) and _p not in sys.path:
        sys.path.append(_p)

import numpy as np

import concourse.bacc as bacc
import concourse.tile as tile
from concourse import mybir
from concourse.bass_utils import run_bass_kernel_spmd
from concourse.masks import make_identity

# Problem shapes (hardcoded per harness contract)
B, S, M, V, KD, FC = 2048, 200, 50, 200, 50, 50
NQ, NQA = 5001, 10001
NCORES = 8
BL = B // NCORES      # 256 batch rows per core
P = 128               # SBUF partitions
NT = BL // P          # 2 batch tiles per core (merged in one state tensor)
KSTEPS = 2            # time steps per gather block
EAW = 640             # ea-table row: (-e)[0:200] | a[200:400] | (-1/e)[400:600]
WHW = 256             # wh-table row: wdup[0:100] | hq[128:178] | w[192:242]
IDX_PER_BLK = BL * KSTEPS
IDXCOLS = BL * S // 16
KC = 4                # MLP chunk length (steps); S % KC == 0
MG = 2                # m-slots per PE accumulation chain group
JA = 7                # tile0 build slots (m >= M-JA) built by ACT, not DVE

_prog_cache = {}


def _build_program(steps=S):
    dt = mybir.dt
    nc = bacc.Bacc("TRN2", debug=False)

    ea_t = nc.dram_tensor("ea_table", [NQA, EAW], dt.float16, kind="ExternalInput")
    wh_t = nc.dram_tensor("wh_table", [NQ, WHW], dt.float16, kind="ExternalInput")
    w1r_d = nc.dram_tensor("w1r", [2, 100, FC], dt.float16, kind="ExternalInput")
    w2_d = nc.dram_tensor("w2rep", [P, FC], dt.float16, kind="ExternalInput")
    b2_d = nc.dram_tensor("b2rep", [P, 1], dt.float32, kind="ExternalInput")
    mv_d = nc.dram_tensor("mv_init", [1, M * V], dt.float16, kind="ExternalInput")
    qi_d = nc.dram_tensor("qidx", [P, IDXCOLS], dt.int16, kind="ExternalInput")
    qa_d = nc.dram_tensor("qaidx", [P, IDXCOLS], dt.int16, kind="ExternalInput")
    preds_d = nc.dram_tensor("preds_out", [BL, S], dt.float32, kind="ExternalOutput")

    nblk = steps // KSTEPS

    from contextlib import ExitStack

    with tile.TileContext(nc) as tc, ExitStack() as ctx:
        consts = ctx.enter_context(tc.tile_pool(name="consts", bufs=1))
        state = ctx.enter_context(tc.tile_pool(name="state", bufs=1))
        gath = ctx.enter_context(tc.tile_pool(name="gath", bufs=2))
        small = ctx.enter_context(tc.tile_pool(name="small", bufs=2))
        psum = ctx.enter_context(tc.tile_pool(name="psum", bufs=2, space="PSUM"))
        psmm = ctx.enter_context(tc.tile_pool(name="psmm", bufs=1, space="PSUM"))
        psrd = ctx.enter_context(tc.tile_pool(name="psrd", bufs=2, space="PSUM"))

        mult = mybir.AluOpType.mult
        addop = mybir.AluOpType.add

        # ---- constants ----
        w1r_sb = consts.tile([100, 2, FC], dt.float16)
        for c in range(2):
            nc.sync.dma_start(out=w1r_sb[:, c, :], in_=w1r_d[c])
        w2_sb = consts.tile([P, FC], dt.float16)
        nc.sync.dma_start(out=w2_sb[:], in_=w2_d[:])
        b2_sb = consts.tile([P, 1], dt.float32)
        nc.sync.dma_start(out=b2_sb[:], in_=b2_d[:])
        ident = consts.tile([P, P], dt.float16)
        make_identity(nc, ident)

        # ---- persistent state ----
        Mv = state.tile([P, NT, M, V], dt.float16, tag="mv", name="Mv")
        for tl in range(NT):
            nc.sync.dma_start(
                out=Mv[:, tl].rearrange("p m v -> p (m v)"),
                in_=mv_d[:].to_broadcast((P, M * V)),
            )
        NW0 = state.tile([P, M, 2 * V], dt.float16, tag="nw0", name="NW0")
        # tile1's field, double-buffered as two SEPARATE tiles: a shared tile
        # serializes the t+1 build behind step t's readers of the other slice
        NW1a = state.tile([P, M, 2 * V], dt.float16, tag="nw1a", name="NW1a")
        NW1b = state.tile([P, M, 2 * V], dt.float16, tag="nw1b", name="NW1b")
        NW1P = [NW1a, NW1b]
        RE = state.tile([P, NT, V], dt.float32, tag="re", name="RE")
        RB = state.tile([P, NT, 2, KC, V], dt.float16, tag="rb", name="RB")
        HQ = state.tile([P, NT, 2, KC, FC], dt.float16, tag="hq", name="HQ")
        RT = state.tile([100, KC, 2, P], dt.float16, tag="rt", name="RT")
        WF = state.tile([P, 2, NT * KSTEPS, M], dt.float32, tag="wf", name="WF")
        preds_buf = state.tile([P, NT, S], dt.float32, tag="preds")

        def flush_pe(c0, klen, cpar):
            """PE part of the batched MLP: transposes + matmuls into PSUM."""
            hps = []
            for tl in range(NT):
                h_ps = psmm.tile([P, KC, FC], dt.float32, tag=f"hps{tl}",
                                 name=f"hps{tl}")
                for kk in range(klen):
                    for h in range(2):
                        pT = psum.tile([100, P], dt.float16, tag="pT", name="pT")
                        nc.tensor.transpose(
                            pT[:], RB[:, tl, cpar, kk, h * 100:(h + 1) * 100],
                            ident[:])
                        nc.scalar.copy(RT[:, kk, h, :], pT[:])
                    nc.tensor.matmul(h_ps[:, kk, :], lhsT=RT[:, kk, 0, :],
                                     rhs=w1r_sb[:, 0, :], start=True, stop=False)
                    nc.tensor.matmul(h_ps[:, kk, :], lhsT=RT[:, kk, 1, :],
                                     rhs=w1r_sb[:, 1, :], start=False, stop=True)
                hps.append(h_ps)
            return hps

        def flush_dve(c0, klen, cpar, hps):
            """DVE/ACT tail of the batched MLP (issued a step later)."""
            for tl in range(NT):
                h_ps = hps[tl]
                hh = small.tile([P, KC, FC], dt.float16, tag="hh", name="hh")
                nc.vector.tensor_add(hh[:, 0:klen, :], h_ps[:, 0:klen, :],
                                     HQ[:, tl, cpar, 0:klen, :])
                nc.scalar.activation(hh[:, 0:klen, :], hh[:, 0:klen, :],
                                     mybir.ActivationFunctionType.Tanh)
                hw2 = small.tile([P, KC, FC], dt.float16, tag="hw2", name="hw2")
                nc.vector.tensor_mul(
                    hw2[:, 0:klen, :], hh[:, 0:klen, :],
                    w2_sb[:, None, :].to_broadcast((P, klen, FC)))
                pacc = small.tile([P, KC], dt.float32, tag="pacc", name="pacc")
                nc.vector.tensor_reduce(pacc[:, 0:klen], hw2[:, 0:klen, :],
                                        mybir.AxisListType.X, addop)
                nc.scalar.activation(
                    preds_buf[:, tl, c0:c0 + klen], pacc[:, 0:klen],
                    mybir.ActivationFunctionType.Sigmoid, bias=b2_sb[:],
                )

        def issue_gather(g):
            qi = gath.tile([P, IDX_PER_BLK // 16], dt.int16, tag="qi", name="qi")
            qa = gath.tile([P, IDX_PER_BLK // 16], dt.int16, tag="qa", name="qa")
            c0 = g * (IDX_PER_BLK // 16)
            nc.sync.dma_start(out=qi[:], in_=qi_d[:, c0:c0 + IDX_PER_BLK // 16])
            nc.sync.dma_start(out=qa[:], in_=qa_d[:, c0:c0 + IDX_PER_BLK // 16])
            ea_blk = gath.tile([P, NT * KSTEPS, EAW], dt.float16, tag="ea", name="ea")
            wh_blk = gath.tile([P, NT * KSTEPS, WHW], dt.float16, tag="wh", name="wh")
            nc.gpsimd.dma_gather(ea_blk[:], ea_t[:], qa[:], IDX_PER_BLK, IDX_PER_BLK, EAW)
            nc.gpsimd.dma_gather(wh_blk[:], wh_t[:], qi[:], IDX_PER_BLK, IDX_PER_BLK, WHW)
            return ea_blk, wh_blk

        def block_prep(g, gpar, ea_blk, wh_blk):
            # fp32 w copies for ACT build scales (all 4 block rows)
            nc.scalar.copy(WF[:, gpar, :, :], wh_blk[:, :, 192:192 + M])

        def act_build_nw1(spar, ea_row, wf_scale):
            """ACT: NW1[spar][m, 0:400] = w[m] * (-e | a) for tile1."""
            for m in range(M):
                nc.scalar.activation(
                    NW1P[spar][:, m, 0:2 * V], ea_row,
                    mybir.ActivationFunctionType.Copy,
                    scale=wf_scale[:, m:m + 1])

        def act_build_nw0j(ea_row, wf_scale):
            """ACT: tile0's last JA field rows (DVE builds the rest)."""
            for m in range(M - JA, M):
                nc.scalar.activation(
                    NW0[:, m, 0:2 * V], ea_row,
                    mybir.ActivationFunctionType.Copy,
                    scale=wf_scale[:, m:m + 1])

        # ---- prologue: first gather + first ACT builds (step 0) ----
        cur = issue_gather(0)
        block_prep(0, 0, *cur)
        act_build_nw1(0, cur[0][:, 1, 0:2 * V], WF[:, 0, 1, :])
        act_build_nw0j(cur[0][:, 0, 0:2 * V], WF[:, 0, 0, :])

        pending = None
        nxt = cur
        for g in range(nblk):
            ea_blk, wh_blk = nxt
            gpar = g % 2
            if g + 1 < nblk:
                nxt = issue_gather(g + 1)

            for k in range(KSTEPS):
                t = g * KSTEPS + k
                kk, cpar, spar = t % KC, (t // KC) % 2, t % 2
                c0t, c1t = 2 * k, 2 * k + 1  # block row ids for tile0/tile1

                # HQ rows for the chunked MLP (both tiles, one ACT op)
                wrows = wh_blk[:].rearrange("p (kx t) w -> p kx t w", t=2)
                nc.scalar.copy(HQ[:, :, cpar, kk, :], wrows[:, k, :, 128:128 + FC])

                # --- DVE: build tile0's NWEA = w (x) (-e | a), m < M-JA ---
                MD = M - JA
                wp = wh_blk[:, c0t, 0:2 * M].rearrange("p (m two) -> p m two", two=2)
                wp_bc = wp[:, 0:MD, None, :].to_broadcast((P, MD, V, 2))
                nea = ea_blk[:, c0t, 0:2 * V].rearrange(
                    "p (vh two) -> p vh two", two=2)
                nea_bc = nea[:, None, :, :].to_broadcast((P, MD, V, 2))
                nw0v = NW0[:, 0:MD, :].rearrange(
                    "p m (vh two) -> p m vh two", two=2)
                nc.vector.tensor_mul(nw0v, wp_bc, nea_bc)

                # --- X2n (in-place over the -w*e fields) ---
                nc.vector.tensor_mul(NW0[:, :, 0:V], NW0[:, :, 0:V], Mv[:, 0])
                nc.vector.tensor_mul(NW1P[spar][:, :, 0:V], NW1P[spar][:, :, 0:V],
                                     Mv[:, 1])

                # --- PE: column-sums of X2n into PSUM (read numerator) ---
                # [P, NT, 512]: each tile's slab fills one PSUM bank exactly;
                # a matmul accumulation target must not straddle banks.
                readp = psrd.tile([P, NT, 512], dt.float32, tag="readp",
                                  name="readp")
                ngrp = M // MG
                for tl in range(NT):
                    src = NW0 if tl == 0 else NW1P[spar]
                    out_v = readp[:, tl, 0:MG * V].rearrange(
                        "p (g v) -> p g v", g=MG)
                    for gi in range(ngrp):
                        nc.tensor.matmul(
                            out_v, lhsT=ident[:],
                            rhs=src[:, MG * gi:MG * (gi + 1), 0:V],
                            start=(gi == 0), stop=(gi == ngrp - 1))

                # --- erase + add (plain flat adds) ---
                nc.vector.tensor_add(Mv[:, 0], Mv[:, 0], NW0[:, :, 0:V])
                nc.vector.tensor_add(Mv[:, 1], Mv[:, 1], NW1P[spar][:, :, 0:V])
                nc.vector.tensor_add(Mv[:, 0], Mv[:, 0], NW0[:, :, V:2 * V])
                nc.vector.tensor_add(Mv[:, 1], Mv[:, 1], NW1P[spar][:, :, V:2 * V])

                # --- ACT: build tile1's NWEA for step t+1 (pingpong) ---
                if t + 1 < steps:
                    kn = (t + 1) % KSTEPS
                    gn = (t + 1) // KSTEPS
                    eab = ea_blk if gn == g else nxt[0]
                    if gn != g:
                        block_prep(gn, gn % 2, *nxt)
                    act_build_nw1(1 - spar, eab[:, 2 * kn + 1, 0:2 * V],
                                  WF[:, gn % 2, 2 * kn + 1, :])
                    act_build_nw0j(eab[:, 2 * kn, 0:2 * V],
                                   WF[:, gn % 2, 2 * kn, :])

                # --- read: reduce PSUM groups + scale by -1/e ---
                rpv = readp[:, :, 0:MG * V].rearrange(
                    "p t (g v) -> p t v g", g=MG)
                nc.vector.tensor_reduce(RE[:], rpv, mybir.AxisListType.X, addop)
                nrec = ea_blk[:].rearrange("p (kx t) w -> p kx t w", t=2)
                nc.vector.tensor_mul(RB[:, :, cpar, kk, :], RE[:],
                                     nrec[:, k, :, 400:400 + V])

                # --- chunked MLP flush ---
                if pending is not None and kk == 2:
                    flush_dve(*pending)
                    pending = None
                if kk == KC - 1:
                    hps = flush_pe(t - KC + 1, KC, cpar)
                    pending = (t - KC + 1, KC, cpar, hps)
        if pending is not None:
            flush_dve(*pending)
            pending = None
        # ---- write out ----
        pv = preds_d[:].rearrange("(n p) s -> n p s", p=P)
        for tl in range(NT):
            nc.sync.dma_start(out=pv[tl][:, 0:steps], in_=preds_buf[:, tl, 0:steps])

    nc.finalize()
    return nc


def _wrap_idx(seq):
    """seq [N] -> [128, N//16] int16 wrapped (idx i at [i%16, i//16], 8x replicated)."""
    n = seq.shape[0]
    arr16 = seq.reshape(n // 16, 16).T.astype(np.int16)
    return np.tile(arr16, (8, 1))


def _host_tables(inputs):
    f32 = np.float32
    qe = inputs["q_embed_w"].astype(f32)
    qae = inputs["qa_embed_w"].astype(f32)
    km = inputs["key_memory"].astype(f32)

    logits = qe @ km.T
    ex = np.exp(logits - logits.max(-1, keepdims=True))
    wsoft = ex / ex.sum(-1, keepdims=True)
    hq = qe @ inputs["pred_w1"][V:, :].astype(f32) + inputs["pred_b1"].astype(f32)
    esig = 1.0 / (1.0 + np.exp(-(qae @ inputs["erase_w"].astype(f32)
                                 + inputs["erase_b"].astype(f32))))
    atanh = np.tanh(qae @ inputs["add_w"].astype(f32) + inputs["add_b"].astype(f32))

    ea = np.zeros((NQA, EAW), np.float16)
    ea[:, 0:V] = (-esig).astype(np.float16)
    ea[:, V:2 * V] = atanh.astype(np.float16)
    ea[:, 400:400 + V] = (-1.0 / esig).astype(np.float16)
    wh = np.zeros((NQ, WHW), np.float16)
    wh[:, 0:2 * M] = np.repeat(wsoft.astype(np.float16), 2, axis=1)
    wh[:, 128:128 + FC] = hq.astype(np.float16)
    wh[:, 192:192 + M] = wsoft.astype(np.float16)

    w1r = inputs["pred_w1"][:V, :].astype(np.float16).reshape(2, 100, FC)
    w2rep = np.tile(inputs["pred_w2"][:, 0].astype(np.float16)[None, :], (P, 1))
    b2rep = np.full((P, 1), inputs["pred_b2"][0], np.float32)
    mv_init = inputs["init_value_memory"].astype(np.float16).reshape(1, -1)
    return dict(ea_table=ea, wh_table=wh, w1r=w1r, w2rep=w2rep, b2rep=b2rep,
                mv_init=mv_init)


def kernel(**inputs):
    inputs = {k: np.asarray(v) for k, v in inputs.items()}
    steps = int(os.environ.get("KERNEL_STEPS", S))

    if steps not in _prog_cache:
        _prog_cache[steps] = _build_program(steps)
    nc = _prog_cache[steps]

    shared = _host_tables(inputs)
    q = inputs["q_data"].astype(np.int64)
    qa = inputs["qa_data"].astype(np.int64)

    in_maps = []
    for core in range(NCORES):
        qs = q[core * BL:(core + 1) * BL]       # [256, S]
        qas = qa[core * BL:(core + 1) * BL]
        # gather order: block g, step k, tile tl, partition p
        def order(x):
            xt = x.T.reshape(S, NT, P)
            return xt.reshape(S // KSTEPS, KSTEPS, NT, P).reshape(-1)
        m = dict(shared)
        m["qidx"] = _wrap_idx(order(qs))
        m["qaidx"] = _wrap_idx(order(qas))
        in_maps.append(m)

    trace = bool(int(os.environ.get("KERNEL_TRACE", "0")))
    res = run_bass_kernel_spmd(nc, in_maps, core_ids=list(range(NCORES)), trace=trace)
    global LAST_RESULTS
    LAST_RESULTS = res
    preds = np.concatenate(
        [res.results[i]["preds_out"] for i in range(NCORES)], axis=0
    ).astype(np.float32)
    z = np.zeros_like(preds)
    return (preds, z, z, z)
